# revision 8
# baseline (speedup 1.0000x reference)
"""Multi-head distance (attention) layer on 8 TRN2 NeuronCores.

Sharding: data-parallel over batch. B=8 -> one batch element per core.
Each core computes a full multi-head self-attention for its [L=1024, D=256]
slice with H=8 heads of dim 64. No collectives needed.

The kernel is ScalarE(ACT)-throughput-bound: softmax needs exp on all
H*L*L = 8.4M scores and ACT is the only engine with exp, at 128 lanes x
1.2 GHz => ~55us floor.  Everything is organized around keeping ACT 100%
busy doing nothing but exp:
  - PSUM: 6 banks are an S-score rotation (2 tiles x [128, 1536] fp32,
    i.e. 3 matmul chunks of 512 per exp) so each ACTIVATE amortizes its
    352-cycle fixed overhead over 1536 elements; the other 2 banks are a
    shared pool for QKV-projection / transpose / O-quad matmul outputs.
  - ACT executes ONLY the exp stream (plus the wk input DMA and one
    table preload, both finished before the first exp); every PSUM
    drain/copy lives on DVE (gpsimd/Pool cannot touch PSUM).
  - Input DMAs are spread over the SP/ACT/Pool queues so the operands on
    the critical path (x, pe, wk, wq) all land within ~2.5us of the
    ~7us fixed engine-sync preamble.
  - S matmuls use 64x128 PE row-tiling (tile_position): head 2j streams
    through PE rows 0-63 while head 2j+1 streams rows 64-127
    concurrently, so the d=64 contraction runs at full rate with no
    zero-padded K and no zeroed kT copies.
Per-core algorithm (all matmul operands fp16: 1 col/cycle on the PE with
~fp32-grade mantissa for this problem's value ranges):
  xT   = transpose(x)            (PE matmul against identity)
  qkT  = xT + peT                (pos-enc, host-precomputed, DVE)
  qT   = Wq.T @ x_pe             via matmul(lhsT=Wq, rhs=qkT)
  kTp  = Wk.T @ x_pe             per head-pair [128=2x64 d, 1024 m]
  v    = x @ Wv                  via matmul(lhsT=xT, rhs=Wv)
  per head pair (2j, 2j+1), interleaved chunk stream:
    sT[m,l] = sum_d kTp[d,m] qT[d,l]     row-tiled matmul chunks
    eT      = exp(0.125 * sT)            ACT, PSUM->SBUF, fp16, [128,1536]
    O[l,d]+Z = eT.T @ [v_h | 1]          matmul(lhsT=eT, rhs=v_aug), 4 output
                                         column-groups share one PSUM bank
    out_h   = O * (1/Z)                  DVE reciprocal + broadcast multiply,
                                         fp16 out_sb, DMA'd out per quad
The S/exp stream is software-pipelined with O(h-2..h-1) so the in-order
PE queue never blocks ACT, which is the pacing engine.
Bias handling: bq added to qT during PSUM drain (per-partition scalar);
bk only shifts each score row by a constant (softmax-invariant) so it is
dropped; bv shifts the output by exactly repeat(bv, 64) because softmax
rows sum to 1, added on the host (which also upcasts the fp16 result).
"""

import numpy as np
import ml_dtypes

import concourse.bass as bass
import concourse.mybir as mybir
import concourse.tile as tile
from concourse import bacc
from concourse.bass_utils import run_bass_kernel_spmd
from concourse.masks import make_identity

B, L, D = 8, 1024, 256
H, HD = 8, 64
J = H * HD  # 512
TEMPERATURE = 10000.0

f32 = mybir.dt.float32
bf16 = mybir.dt.float16  # fp16: same PE rate as bf16, 8x the mantissa

_CACHE = {}
LAST_RESULT = None  # BassKernelResults of the most recent run (for profiling)
TRACE = False

STILE = 1536  # S-chunk PSUM/exp tile width (3 chunks of 512)


def _emit(tc, aps):
    nc = tc.nc
    Exp = mybir.ActivationFunctionType.Exp
    x, wq, wk, wv, bqc, pet, out = (
        aps["x"], aps["wq"], aps["wk"], aps["wv"], aps["bqc"], aps["pet"], aps["out"],
    )

    xr = x.rearrange("(n p) c -> p n c", p=128)          # [128, 8, 256]
    petr = pet.rearrange("(t p) l -> t p l", p=128)      # [2, 128, 1024]
    wqr = wq.rearrange("(t p) j -> t p j", p=128)        # [2, 128, 512]
    wkr = wk.rearrange("(t p) j -> t p j", p=128)
    wvr = wv.rearrange("(t p) j -> t p j", p=128)
    outr = out.rearrange("(n p) j -> p n j", p=128)      # [128, 8, 512]

    import contextlib
    ctx = contextlib.ExitStack()
    persist = ctx.enter_context(tc.tile_pool(name="persist", bufs=1))
    epool = ctx.enter_context(tc.tile_pool(name="epool", bufs=16))
    rpool = ctx.enter_context(tc.tile_pool(name="rpool", bufs=4))
    s_ps = ctx.enter_context(tc.tile_pool(name="sps", bufs=2, space="PSUM"))
    o_ps = ctx.enter_context(tc.tile_pool(name="ops", bufs=2, space="PSUM"))

    # identity for PE transposes first on the Pool queue, ahead of its DMAs
    ident = persist.tile([128, 128], bf16, name="ident")
    make_identity(nc, ident)

    # --- input DMAs, spread so critical-path operands land first:
    #   SP:   x (4 chunks), wq half 0
    #   ACT:  wk (idle until the first exp, which transitively needs wk)
    #   Pool: pe, wq half 1, bq, wv
    x_sb = persist.tile([128, 8, 256], bf16, name="x_sb")
    for qtr in range(4):
        nc.sync.dma_start(out=x_sb[:, qtr * 2:(qtr + 1) * 2, :],
                          in_=xr[:, qtr * 2:(qtr + 1) * 2, :])
    w_sb = {}
    for wname in ("wq", "wk", "wv"):
        w_sb[wname] = [
            persist.tile([128, 512], bf16, name=f"{wname}_sb{t}") for t in range(2)
        ]
    nc.sync.dma_start(out=w_sb["wq"][0][:], in_=wqr[0])
    for t in range(2):
        nc.scalar.dma_start(out=w_sb["wk"][t][:], in_=wkr[t])

    pe_sb = [persist.tile([128, 1024], bf16, name=f"pe_sb{t}") for t in range(2)]
    for t in range(2):
        nc.gpsimd.dma_start(out=pe_sb[t][:], in_=petr[t])
    nc.gpsimd.dma_start(out=w_sb["wq"][1][:], in_=wqr[1])
    bq_sb = persist.tile([128, 4], f32, name="bq_sb")
    nc.gpsimd.dma_start(out=bq_sb[:], in_=bqc[:, :])
    for t in range(2):
        nc.gpsimd.dma_start(out=w_sb["wv"][t][:], in_=wvr[t])

    # --- ACT exp-table preload (after wk DMAs, before the first exp) ---
    sc_in = persist.tile([128, 8], f32, name="sc_in")
    sc_out = persist.tile([128, 8], f32, name="sc_out")
    nc.vector.memset(sc_in[:], 0.0)
    nc.scalar.activation(sc_out[:], sc_in[:], Exp)

    # --- transpose x via PE (out = x_chunk.T @ I); 4 transposes packed per
    # o-pool PSUM tile, drained by DVE ---
    xT = [persist.tile([128, 1024], bf16, name=f"xT{t}") for t in range(2)]
    for g in range(2):  # n-groups of 4
        for c2 in range(2):
            tp = o_ps.tile([128, 512], f32, tag="o", name="tp")
            for i in range(4):
                n = 4 * g + i
                nc.tensor.matmul(
                    tp[:, i * 128:(i + 1) * 128],
                    lhsT=x_sb[:, n, c2 * 128:(c2 + 1) * 128],
                    rhs=ident[:],
                    start=True,
                    stop=True,
                )
            dst = xT[c2][:, g * 512:(g + 1) * 512]
            nc.vector.tensor_copy(dst, tp[:, 0:512])

    # (b) qkT adds split per (c2, l-half) so each fires as soon as its
    # transpose-drain half lands
    qkT = [persist.tile([128, 1024], bf16, name=f"qkT{t}") for t in range(2)]
    for g in range(2):
        for t in range(2):
            sl = slice(g * 512, (g + 1) * 512)
            nc.vector.tensor_add(qkT[t][:, sl], xT[t][:, sl], pe_sb[t][:, sl])

    # --- QKV projections (o-pool PSUM, drains on DVE) ---
    kTp = [persist.tile([128, 1024], bf16, name=f"kTp{j}") for j in range(4)]
    qT = [persist.tile([128, 1024], bf16, name=f"qT{j}") for j in range(4)]
    v_sb = [persist.tile([128, 8, 65], bf16, name=f"v_sb{m}") for m in range(8)]

    def qk_piece(j, which, l2):
        wname = "wq" if which == "q" else "wk"
        pq = o_ps.tile([128, 512], f32, tag="o", name="pq")
        for c2 in range(2):
            nc.tensor.matmul(
                pq[:, 0:512],
                lhsT=w_sb[wname][c2][:, j * 128:(j + 1) * 128],
                rhs=qkT[c2][:, l2 * 512:(l2 + 1) * 512],
                start=(c2 == 0),
                stop=(c2 == 1),
            )
        dsl = slice(l2 * 512, (l2 + 1) * 512)
        if which == "q":
            nc.vector.tensor_scalar_add(
                qT[j][:, dsl], pq[:, 0:512], bq_sb[:, j:j + 1]
            )
        else:
            nc.vector.tensor_copy(kTp[j][:, dsl], pq[:, 0:512])

    def v_proj(m):
        pv = o_ps.tile([128, 512], f32, tag="o", name="pv")
        for c2 in range(2):
            nc.tensor.matmul(
                pv[:, 0:512],
                lhsT=xT[c2][:, m * 128:(m + 1) * 128],
                rhs=w_sb["wv"][c2][:],
                start=(c2 == 0),
                stop=(c2 == 1),
            )
        nc.vector.tensor_copy(
            v_sb[m][:, :, 0:64], pv[:, 0:512].rearrange("p (h d) -> p h d", h=8)
        )
        nc.vector.memset(v_sb[m][:, :, 64:65], 1.0)

    # --- attention: S-chunks packed into [128, STILE] PSUM tiles; one exp
    # per tile. Software-pipelined: S(pair p) emitted before O(2p-2..2p-1). ---
    out_sb = persist.tile([128, 8, 512], bf16, name="out_sb")
    epos = {}  # (h, mc, l2) -> (e_tile, col_offset)
    state = {"tile": None, "off": 0, "chunks": []}

    def flush_exp():
        if state["tile"] is None or not state["chunks"]:
            return
        e = epool.tile([128, state["off"]], bf16, tag="e", name="e")
        nc.scalar.activation(
            e[:], state["tile"][:, 0:state["off"]], Exp, scale=float(HD) ** -0.5
        )
        for key, off in state["chunks"]:
            epos[key] = (e, off)
        state["tile"] = None
        state["off"] = 0
        state["chunks"] = []

    def s_chunk(h, mc, l2):
        if state["tile"] is None:
            state["tile"] = s_ps.tile([128, STILE], f32, tag="s", name="ps")
        off = state["off"]
        j, half = h >> 1, h & 1
        hp = slice(half * 64, half * 64 + 64)
        nc.tensor.matmul(
            state["tile"][:, off:off + 512],
            lhsT=kTp[j][hp, mc * 128:(mc + 1) * 128],
            rhs=qT[j][hp, l2 * 512:(l2 + 1) * 512],
            start=True,
            stop=True,
            tile_position=(half * 64, 0),
        )
        state["chunks"].append(((h, mc, l2), off))
        state["off"] = off + 512
        if state["off"] == STILE:
            flush_exp()

    def emit_S_pair(p, l2):
        for mc in range(8):
            s_chunk(2 * p, mc, l2)
            s_chunk(2 * p + 1, mc, l2)

    def emit_O_quad(h, q):
        hsl = slice(h * 64, (h + 1) * 64)
        pO = o_ps.tile([128, 260], f32, tag="o", name="pO")
        for g in range(4):
            lc = 4 * q + g
            l2, sub = lc // 4, lc % 4
            for mc in range(8):
                e, off = epos[(h, mc, l2)]
                nc.tensor.matmul(
                    pO[:, 65 * g:65 * g + 65],
                    lhsT=e[:, off + sub * 128:off + (sub + 1) * 128],
                    rhs=v_sb[mc][:, h, :],
                    start=(mc == 0),
                    stop=(mc == 7),
                )
        pOr = pO.rearrange("p (g c) -> p g c", g=4)      # [128, 4, 65]
        rc = rpool.tile([128, 4], f32, tag="rc", name="rc")
        nc.vector.reciprocal(rc[:], pOr[:, :, 64])
        rcb = bass.AP(
            tensor=rc.tensor, offset=rc.offset,
            ap=[rc.ap[0], rc.ap[1], [0, 64]],
        )
        nc.vector.tensor_mul(
            out_sb[:, 4 * q:4 * q + 4, hsl], pOr[:, :, 0:64], rcb
        )
        if h == 7:
            engs = [nc.sync, nc.gpsimd, nc.scalar, nc.sync] if q == 1 else [
                nc.sync, nc.gpsimd, nc.sync, nc.gpsimd]
            for g2 in range(4):
                sl2 = slice(4 * q + g2, 4 * q + g2 + 1)
                engs[g2].dma_start(out=outr[:, sl2, hsl], in_=out_sb[:, sl2, hsl])
        else:
            eng = nc.sync if q == 0 else nc.gpsimd
            eng.dma_start(
                out=outr[:, 4 * q:4 * q + 4, hsl],
                in_=out_sb[:, 4 * q:4 * q + 4, hsl],
            )

    # schedule in head-pair blocks: S-pair emissions and O emissions for the
    # previous pair's heads are interleaved; QKV projections for pair p+1 are
    # dropped in between steps of block p so they never bunch up in front of
    # an S-fill.
    qk_piece(0, "k", 0)
    qk_piece(0, "k", 1)
    qk_piece(0, "q", 0)
    emit_S_pair(0, 0)
    qk_piece(0, "q", 1)
    qk_piece(1, "k", 0)
    qk_piece(1, "k", 1)
    for m in range(4):
        v_proj(m)
    emit_S_pair(0, 1)
    qk_piece(1, "q", 0)
    qk_piece(1, "q", 1)
    for m in range(4, 8):
        v_proj(m)
    for p in range(1, 4):
        steps = [
            ("S", p, 0), ("O", 2 * p - 2, 0), ("O", 2 * p - 2, 1),
            ("S", p, 1), ("O", 2 * p - 1, 0), ("O", 2 * p - 1, 1),
        ]
        inject = [("k", 0), ("k", 1), ("q", 0), ("q", 1)]
        for stepi, (kind, a, b) in enumerate(steps):
            if kind == "S":
                emit_S_pair(a, b)
            else:
                emit_O_quad(a, b)
            if p < 3 and stepi < 4:
                w, l2 = inject[stepi]
                qk_piece(p + 1, w, l2)
    flush_exp()
    emit_O_quad(6, 0)
    emit_O_quad(6, 1)
    emit_O_quad(7, 0)
    emit_O_quad(7, 1)
    ctx.close()


def _build():
    if "nc" in _CACHE:
        return _CACHE["nc"]
    nc = bacc.Bacc("TRN2", target_bir_lowering=False, debug=False, num_devices=8)
    aps = {
        "x": nc.dram_tensor("x", [L, D], bf16, kind="ExternalInput").ap(),
        "wq": nc.dram_tensor("wq", [D, J], bf16, kind="ExternalInput").ap(),
        "wk": nc.dram_tensor("wk", [D, J], bf16, kind="ExternalInput").ap(),
        "wv": nc.dram_tensor("wv", [D, J], bf16, kind="ExternalInput").ap(),
        "bqc": nc.dram_tensor("bqc", [128, 4], f32, kind="ExternalInput").ap(),
        "pet": nc.dram_tensor("pet", [D, L], bf16, kind="ExternalInput").ap(),
        "out": nc.dram_tensor("out", [L, J], bf16, kind="ExternalOutput").ap(),
    }
    with tile.TileContext(nc) as tc:
        _emit(tc, aps)
    nc.compile()
    _CACHE["nc"] = nc
    return nc


def _pe_T():
    embed = np.arange(L, dtype=np.float32)
    dim_t = np.arange(D, dtype=np.float32)
    dim_t = (np.float32(TEMPERATURE) ** (2.0 * np.floor(dim_t / 2.0) / np.float32(D))).astype(np.float32)
    pos = embed[:, None] / dim_t  # [L, D]
    pe = np.stack([np.sin(pos[:, 0::2]), np.cos(pos[:, 1::2])], axis=2).reshape(L, D)
    return np.ascontiguousarray(pe.T.astype(np.float32))  # [D, L]


def kernel(**inputs):
    global LAST_RESULT
    bf = np.float16
    x = np.asarray(inputs["x"], dtype=np.float32).astype(bf)
    wq = np.ascontiguousarray(np.asarray(inputs["Wq"], dtype=np.float32).astype(bf))
    wk = np.ascontiguousarray(np.asarray(inputs["Wk"], dtype=np.float32).astype(bf))
    wv = np.ascontiguousarray(np.asarray(inputs["Wv"], dtype=np.float32).astype(bf))
    bq = np.asarray(inputs["bq"], dtype=np.float32)
    bv = np.asarray(inputs["bv"], dtype=np.float32)

    nc = _build()
    bqc = np.ascontiguousarray(np.repeat(bq, HD).reshape(4, 128).T)  # [128, 4]
    pet = _pe_T().astype(bf)
    base = {"wq": wq, "wk": wk, "wv": wv, "bqc": bqc, "pet": pet}
    in_maps = [{**base, "x": np.ascontiguousarray(x[b])} for b in range(B)]
    res = run_bass_kernel_spmd(
        nc, in_maps, core_ids=list(range(B)), trace=TRACE
    )
    LAST_RESULT = res
    out = np.stack([res.results[b]["out"] for b in range(B)]).astype(np.float32)
    out += np.repeat(bv, HD)[None, None, :]
    return out


# revision 9
# speedup vs baseline: 1.1148x; 1.1148x over previous
"""Multi-head distance (attention) layer on 8 TRN2 NeuronCores.

Sharding: data-parallel over batch. B=8 -> one batch element per core.
Each core computes a full multi-head self-attention for its [L=1024, D=256]
slice with H=8 heads of dim 64. No collectives needed.

The kernel is ScalarE(ACT)-throughput-bound: softmax needs exp on all
H*L*L = 8.4M scores and ACT is the only engine with exp, at 128 lanes x
1.2 GHz => ~55us floor.  Everything is organized around keeping ACT 100%
busy doing nothing but exp:
  - The host ships x already TRANSPOSED, both with the positional
    encoding added in fp32 (qkpT = (x+pe).T, feeds Q/K) and without
    (xT, feeds V).  That kills the on-device transposes and pos-enc
    adds and makes every input DMA a single large contiguous transfer
    (one InstDMACopy stripes across all 16 SDMA engines).
  - PSUM: 6 banks are an S-score rotation (2 tiles x [128, 1536] fp32,
    i.e. 3 matmul chunks of 512 per exp) so each ACTIVATE amortizes its
    352-cycle fixed overhead over 1536 elements; the other 2 banks are a
    shared pool for QKV-projection / O-quad matmul outputs.
  - ACT executes ONLY the exp stream (plus input DMA triggers and one
    table preload, all finished before the first exp); every PSUM
    drain/copy lives on DVE (gpsimd/Pool cannot touch PSUM).
  - S matmuls use 64x128 PE row-tiling (tile_position): head 2j streams
    through PE rows 0-63 while head 2j+1 streams rows 64-127
    concurrently, so the d=64 contraction runs at full rate with no
    zero-padded K and no zeroed kT copies.
  - The S/exp stream is software-pipelined with the O quads of the
    previous head pair at half-S-pair granularity so the in-order PE
    queue never runs ACT dry; only the last two O quads (which consume
    the final exps) trail the exp stream.
Per-core algorithm (all matmul operands fp16: 1 col/cycle on the PE with
~fp32-grade mantissa for this problem's value ranges):
  qT   = Wq.T @ qkpT + bq       via matmul(lhsT=Wq, rhs=qkpT), DVE drain
  kTp  = Wk.T @ qkpT            per head-pair [128=2x64 d, 1024 m]
  v    = xT.T @ Wv              via matmul(lhsT=xT, rhs=Wv)
  per head pair (2j, 2j+1), interleaved chunk stream:
    sT[m,l] = sum_d kTp[d,m] qT[d,l]     row-tiled matmul chunks
    eT      = exp(0.125 * sT)            ACT, PSUM->SBUF, fp16, [128,1536]
    O[l,d]+Z = eT.T @ [v_h | 1]          matmul(lhsT=eT, rhs=v_aug), 4 output
                                         column-groups share one PSUM bank
    out_h   = O * (1/Z)                  DVE reciprocal + broadcast multiply,
                                         fp16 out_sb, DMA'd out per quad
Bias handling: bq added to qT during PSUM drain (per-partition scalar);
bk only shifts each score row by a constant (softmax-invariant) so it is
dropped; bv shifts the output by exactly repeat(bv, 64) because softmax
rows sum to 1, added on the host (which also upcasts the fp16 result).
"""

import numpy as np

import concourse.bass as bass
import concourse.mybir as mybir
import concourse.tile as tile
from concourse import bacc
from concourse.bass_utils import run_bass_kernel_spmd

B, L, D = 8, 1024, 256
H, HD = 8, 64
J = H * HD  # 512
TEMPERATURE = 10000.0

f32 = mybir.dt.float32
bf16 = mybir.dt.float16  # fp16: same PE rate as bf16, 8x the mantissa

_CACHE = {}
LAST_RESULT = None  # BassKernelResults of the most recent run (for profiling)
TRACE = False

STILE = 1536  # S-chunk PSUM/exp tile width (3 chunks of 512)


def _emit(tc, aps):
    nc = tc.nc
    Exp = mybir.ActivationFunctionType.Exp
    qkp, xt, wq, wk, wv, bqc, out = (
        aps["qkp"], aps["xt"], aps["wq"], aps["wk"], aps["wv"], aps["bqc"],
        aps["out"],
    )

    qkpr = qkp.rearrange("(t p) l -> t p l", p=128)      # [2, 128, 1024]
    xtr = xt.rearrange("(t p) l -> t p l", p=128)        # [2, 128, 1024]
    wqr = wq.rearrange("(t p) j -> t p j", p=128)        # [2, 128, 512]
    wkr = wk.rearrange("(t p) j -> t p j", p=128)
    wvr = wv.rearrange("(t p) j -> t p j", p=128)
    outr = out.rearrange("(n p) j -> p n j", p=128)      # [128, 8, 512]

    import contextlib
    ctx = contextlib.ExitStack()
    persist = ctx.enter_context(tc.tile_pool(name="persist", bufs=1))
    epool = ctx.enter_context(tc.tile_pool(name="epool", bufs=16))
    rpool = ctx.enter_context(tc.tile_pool(name="rpool", bufs=4))
    s_ps = ctx.enter_context(tc.tile_pool(name="sps", bufs=2, space="PSUM"))
    o_ps = ctx.enter_context(tc.tile_pool(name="ops", bufs=2, space="PSUM"))

    # --- input DMAs: one large contiguous transfer each, spread over the
    # three DMA-capable queues, critical-path operands (qkp, wk, wq) first.
    qkT = [persist.tile([128, 1024], bf16, name=f"qkT{t}") for t in range(2)]
    xT = [persist.tile([128, 1024], bf16, name=f"xT{t}") for t in range(2)]
    w_sb = {}
    for wname in ("wq", "wk", "wv"):
        w_sb[wname] = [
            persist.tile([128, 512], bf16, name=f"{wname}_sb{t}") for t in range(2)
        ]
    bq_sb = persist.tile([128, 4], f32, name="bq_sb")

    nc.sync.dma_start(out=qkT[0][:], in_=qkpr[0])
    nc.scalar.dma_start(out=qkT[1][:], in_=qkpr[1])
    nc.gpsimd.dma_start(out=w_sb["wk"][0][:], in_=wkr[0])
    nc.gpsimd.dma_start(out=w_sb["wk"][1][:], in_=wkr[1])
    nc.sync.dma_start(out=w_sb["wq"][0][:], in_=wqr[0])
    nc.scalar.dma_start(out=w_sb["wq"][1][:], in_=wqr[1])
    nc.gpsimd.dma_start(out=bq_sb[:], in_=bqc[:, :])
    nc.sync.dma_start(out=xT[0][:], in_=xtr[0])
    nc.scalar.dma_start(out=xT[1][:], in_=xtr[1])
    nc.gpsimd.dma_start(out=w_sb["wv"][0][:], in_=wvr[0])
    nc.gpsimd.dma_start(out=w_sb["wv"][1][:], in_=wvr[1])

    # --- ACT exp-table preload (after ACT's DMA triggers, before first exp)
    sc_in = persist.tile([128, 8], f32, name="sc_in")
    sc_out = persist.tile([128, 8], f32, name="sc_out")
    nc.vector.memset(sc_in[:], 0.0)
    nc.scalar.activation(sc_out[:], sc_in[:], Exp)

    # --- QKV projections (o-pool PSUM, drains on DVE) ---
    kTp = [persist.tile([128, 1024], bf16, name=f"kTp{j}") for j in range(4)]
    qT = [persist.tile([128, 1024], bf16, name=f"qT{j}") for j in range(4)]
    v_sb = [persist.tile([128, 8, 65], bf16, name=f"v_sb{m}") for m in range(8)]

    def qk_piece(j, which, l2):
        wname = "wq" if which == "q" else "wk"
        pq = o_ps.tile([128, 512], f32, tag="o", name="pq")
        for c2 in range(2):
            nc.tensor.matmul(
                pq[:, 0:512],
                lhsT=w_sb[wname][c2][:, j * 128:(j + 1) * 128],
                rhs=qkT[c2][:, l2 * 512:(l2 + 1) * 512],
                start=(c2 == 0),
                stop=(c2 == 1),
            )
        dsl = slice(l2 * 512, (l2 + 1) * 512)
        if which == "q":
            nc.vector.tensor_scalar_add(
                qT[j][:, dsl], pq[:, 0:512], bq_sb[:, j:j + 1]
            )
        else:
            nc.vector.tensor_copy(kTp[j][:, dsl], pq[:, 0:512])

    def v_proj(m):
        pv = o_ps.tile([128, 512], f32, tag="o", name="pv")
        for c2 in range(2):
            nc.tensor.matmul(
                pv[:, 0:512],
                lhsT=xT[c2][:, m * 128:(m + 1) * 128],
                rhs=w_sb["wv"][c2][:],
                start=(c2 == 0),
                stop=(c2 == 1),
            )
        nc.vector.tensor_copy(
            v_sb[m][:, :, 0:64], pv[:, 0:512].rearrange("p (h d) -> p h d", h=8)
        )
        nc.vector.memset(v_sb[m][:, :, 64:65], 1.0)

    # --- attention: S-chunks packed into [128, STILE] PSUM tiles; one exp
    # per tile. ---
    out_sb = persist.tile([128, 8, 512], bf16, name="out_sb")
    epos = {}  # (h, mc, l2) -> (e_tile, col_offset)
    state = {"tile": None, "off": 0, "chunks": []}

    def flush_exp():
        if state["tile"] is None or not state["chunks"]:
            return
        e = epool.tile([128, state["off"]], bf16, tag="e", name="e")
        nc.scalar.activation(
            e[:], state["tile"][:, 0:state["off"]], Exp, scale=float(HD) ** -0.5
        )
        for key, off in state["chunks"]:
            epos[key] = (e, off)
        state["tile"] = None
        state["off"] = 0
        state["chunks"] = []

    def s_chunk(h, mc, l2):
        if state["tile"] is None:
            state["tile"] = s_ps.tile([128, STILE], f32, tag="s", name="ps")
        off = state["off"]
        j, half = h >> 1, h & 1
        hp = slice(half * 64, half * 64 + 64)
        nc.tensor.matmul(
            state["tile"][:, off:off + 512],
            lhsT=kTp[j][hp, mc * 128:(mc + 1) * 128],
            rhs=qT[j][hp, l2 * 512:(l2 + 1) * 512],
            start=True,
            stop=True,
            tile_position=(half * 64, 0),
        )
        state["chunks"].append(((h, mc, l2), off))
        state["off"] = off + 512
        if state["off"] == STILE:
            flush_exp()

    def emit_S_ph(p, l2, half4):
        for mc in (range(4) if half4 == 0 else range(4, 8)):
            s_chunk(2 * p, mc, l2)
            s_chunk(2 * p + 1, mc, l2)

    def emit_O_quad(h, q):
        hsl = slice(h * 64, (h + 1) * 64)
        pO = o_ps.tile([128, 260], f32, tag="o", name="pO")
        for g in range(4):
            lc = 4 * q + g
            l2, sub = lc // 4, lc % 4
            for mc in range(8):
                e, off = epos[(h, mc, l2)]
                nc.tensor.matmul(
                    pO[:, 65 * g:65 * g + 65],
                    lhsT=e[:, off + sub * 128:off + (sub + 1) * 128],
                    rhs=v_sb[mc][:, h, :],
                    start=(mc == 0),
                    stop=(mc == 7),
                )
        pOr = pO.rearrange("p (g c) -> p g c", g=4)      # [128, 4, 65]
        rc = rpool.tile([128, 4], f32, tag="rc", name="rc")
        nc.vector.reciprocal(rc[:], pOr[:, :, 64])
        rcb = bass.AP(
            tensor=rc.tensor, offset=rc.offset,
            ap=[rc.ap[0], rc.ap[1], [0, 64]],
        )
        nc.vector.tensor_mul(
            out_sb[:, 4 * q:4 * q + 4, hsl], pOr[:, :, 0:64], rcb
        )
        if h == 7:
            engs = [nc.sync, nc.gpsimd, nc.scalar, nc.sync] if q == 1 else [
                nc.sync, nc.gpsimd, nc.sync, nc.gpsimd]
            for g2 in range(4):
                sl2 = slice(4 * q + g2, 4 * q + g2 + 1)
                engs[g2].dma_start(out=outr[:, sl2, hsl], in_=out_sb[:, sl2, hsl])
        else:
            eng = nc.sync if q == 0 else nc.gpsimd
            eng.dma_start(
                out=outr[:, 4 * q:4 * q + 4, hsl],
                in_=out_sb[:, 4 * q:4 * q + 4, hsl],
            )

    # schedule: S emission in half-S-pair steps (8 chunks) with the previous
    # pair's O quads and the next pair's QKV projections dropped in between,
    # so the in-order PE queue always has S fills near the head and ACT never
    # runs dry.  The last pair's l2=0 O quads are pulled forward so only
    # O(6,1)/O(7,1) trail the final exp.
    qk_piece(0, "k", 0)
    qk_piece(0, "k", 1)
    qk_piece(0, "q", 0)
    emit_S_ph(0, 0, 0)
    qk_piece(0, "q", 1)
    emit_S_ph(0, 0, 1)
    qk_piece(1, "k", 0)
    emit_S_ph(0, 1, 0)
    qk_piece(1, "k", 1)
    v_proj(0)
    v_proj(1)
    emit_S_ph(0, 1, 1)
    qk_piece(1, "q", 0)
    qk_piece(1, "q", 1)
    for m in range(2, 8):
        v_proj(m)
    for p in range(1, 4):
        if p < 3:
            oslots = [
                [(2 * p - 2, 0)], [(2 * p - 2, 1)], [(2 * p - 1, 0)],
                [(2 * p - 1, 1)],
            ]
            inj = [("k", 0), ("k", 1), ("q", 0), ("q", 1)]
        else:
            oslots = [[(4, 0)], [(4, 1)], [(5, 0), (5, 1), (6, 0), (7, 0)], []]
            inj = None
        for stepi, (l2, half4) in enumerate([(0, 0), (0, 1), (1, 0), (1, 1)]):
            emit_S_ph(p, l2, half4)
            for h, q in oslots[stepi]:
                emit_O_quad(h, q)
            if inj is not None:
                w, l2i = inj[stepi]
                qk_piece(p + 1, w, l2i)
    flush_exp()
    emit_O_quad(6, 1)
    emit_O_quad(7, 1)
    ctx.close()


def _build():
    if "nc" in _CACHE:
        return _CACHE["nc"]
    nc = bacc.Bacc("TRN2", target_bir_lowering=False, debug=False, num_devices=8)
    aps = {
        "qkp": nc.dram_tensor("qkp", [D, L], bf16, kind="ExternalInput").ap(),
        "xt": nc.dram_tensor("xt", [D, L], bf16, kind="ExternalInput").ap(),
        "wq": nc.dram_tensor("wq", [D, J], bf16, kind="ExternalInput").ap(),
        "wk": nc.dram_tensor("wk", [D, J], bf16, kind="ExternalInput").ap(),
        "wv": nc.dram_tensor("wv", [D, J], bf16, kind="ExternalInput").ap(),
        "bqc": nc.dram_tensor("bqc", [128, 4], f32, kind="ExternalInput").ap(),
        "out": nc.dram_tensor("out", [L, J], bf16, kind="ExternalOutput").ap(),
    }
    with tile.TileContext(nc) as tc:
        _emit(tc, aps)
    nc.compile()
    _CACHE["nc"] = nc
    return nc


def _pe():
    embed = np.arange(L, dtype=np.float32)
    dim_t = np.arange(D, dtype=np.float32)
    dim_t = (np.float32(TEMPERATURE) ** (2.0 * np.floor(dim_t / 2.0) / np.float32(D))).astype(np.float32)
    pos = embed[:, None] / dim_t  # [L, D]
    return np.stack([np.sin(pos[:, 0::2]), np.cos(pos[:, 1::2])], axis=2).reshape(L, D)


def kernel(**inputs):
    global LAST_RESULT
    bf = np.float16
    x = np.asarray(inputs["x"], dtype=np.float32)
    wq = np.ascontiguousarray(np.asarray(inputs["Wq"], dtype=np.float32).astype(bf))
    wk = np.ascontiguousarray(np.asarray(inputs["Wk"], dtype=np.float32).astype(bf))
    wv = np.ascontiguousarray(np.asarray(inputs["Wv"], dtype=np.float32).astype(bf))
    bq = np.asarray(inputs["bq"], dtype=np.float32)
    bv = np.asarray(inputs["bv"], dtype=np.float32)

    nc = _build()
    bqc = np.ascontiguousarray(np.repeat(bq, HD).reshape(4, 128).T)  # [128, 4]
    pe = _pe()  # [L, D] fp32
    qkp_all = (x + pe[None]).transpose(0, 2, 1).astype(bf)   # [B, D, L]
    xt_all = x.transpose(0, 2, 1).astype(bf)                 # [B, D, L]
    base = {"wq": wq, "wk": wk, "wv": wv, "bqc": bqc}
    in_maps = [
        {
            **base,
            "qkp": np.ascontiguousarray(qkp_all[b]),
            "xt": np.ascontiguousarray(xt_all[b]),
        }
        for b in range(B)
    ]
    res = run_bass_kernel_spmd(
        nc, in_maps, core_ids=list(range(B)), trace=TRACE
    )
    LAST_RESULT = res
    out = np.stack([res.results[b]["out"] for b in range(B)]).astype(np.float32)
    out += np.repeat(bv, HD)[None, None, :]
    return out
